# revision 9
# baseline (speedup 1.0000x reference)
"""Trainium2 Bass kernel for the DVAE problem.

Full-input contract: kernel(**inputs) takes the complete (unsharded) numpy
inputs and returns (node_logits, edge_probs, mu, logvar) as full numpy arrays.
Internally shards batch 64 -> 8 cores (data parallel), weights replicated.

Math (see reference):
  h      = relu(x @ W1 + b1) @ W2 + b2          [B,N,H]
  hp     = mean(h, axis=1)                      [B,H]
  mu/lv  = hp @ {mu_w,lv_w} + {mu_b,lv_b}       [B,L]
  z      = mu + eps * exp(0.5 lv)
  hd     = relu(z @ z_w + z_b)                  [B,H]
  nlrow  = hd @ nd_w + nd_b                     [B,T]  (broadcast over N)
  e      = sigmoid(relu([hd,hd] @ e1_w + e1_b) @ e2_w + e2_b)   [B]
  edge   = e[:,None,None] * strict_lower(N)     [B,N,N]

Key restructurings:
  - adj_matrix is unused by the math: never shipped or read.
  - mean commutes with the second GNN linear: pool relu(xW1+b1) first, then
    a tiny [B,H]x[H,H] matmul (W2 scaled by 1/N on device).
  - GEMM computed transposed ([h, rows] tiles) so bias is per-partition and
    the node-pool is a free ACT accum_out.
"""

import sys

sys.path.insert(0, "/opt/trn_rl_repo")

from contextlib import ExitStack

import numpy as np

import concourse.bass as bass
from concourse import bacc
import concourse.mybir as mybir
import concourse.tile as tile
from concourse.bass import ds
from concourse.bass_utils import run_bass_kernel_spmd
from concourse.masks import make_identity

F32 = mybir.dt.float32
AF = mybir.ActivationFunctionType

B, N, H, L, T = 64, 1024, 512, 256, 32
NCORES = 8
BPC = B // NCORES  # batches per core = 8
ROWS = BPC * N  # 8192 rows per core
NG = ROWS // 512  # 16 row-groups of 512 rows
NPAIRS = BPC // 2  # chain granularity: 2 batches


def _build():
    nc = bacc.Bacc(None, target_bir_lowering=False)

    x_i = nc.declare_dram_parameter("node_features", [BPC, N, H], F32, isOutput=False)
    eps_i = nc.declare_dram_parameter("eps", [BPC, L], F32, isOutput=False)
    w1_i = nc.declare_dram_parameter("gnn_w1", [H, H], F32, isOutput=False)
    b1_i = nc.declare_dram_parameter("gnn_b1", [H], F32, isOutput=False)
    w2_i = nc.declare_dram_parameter("gnn_w2", [H, H], F32, isOutput=False)
    b2_i = nc.declare_dram_parameter("gnn_b2", [H], F32, isOutput=False)
    muw_i = nc.declare_dram_parameter("mu_w", [H, L], F32, isOutput=False)
    mub_i = nc.declare_dram_parameter("mu_b", [L], F32, isOutput=False)
    lvw_i = nc.declare_dram_parameter("lv_w", [H, L], F32, isOutput=False)
    lvb_i = nc.declare_dram_parameter("lv_b", [L], F32, isOutput=False)
    zw_i = nc.declare_dram_parameter("z_w", [L, H], F32, isOutput=False)
    zb_i = nc.declare_dram_parameter("z_b", [H], F32, isOutput=False)
    ndw_i = nc.declare_dram_parameter("nd_w", [H, T], F32, isOutput=False)
    ndb_i = nc.declare_dram_parameter("nd_b", [T], F32, isOutput=False)
    e1w_i = nc.declare_dram_parameter("e1_w", [2 * H, H], F32, isOutput=False)
    e1b_i = nc.declare_dram_parameter("e1_b", [H], F32, isOutput=False)
    e2w_i = nc.declare_dram_parameter("e2_w", [H, 1], F32, isOutput=False)
    e2b_i = nc.declare_dram_parameter("e2_b", [1], F32, isOutput=False)

    nl_o = nc.declare_dram_parameter("node_logits", [BPC, N, T], F32, isOutput=True)
    ep_o = nc.declare_dram_parameter("edge_probs", [BPC, N, N], F32, isOutput=True)
    mu_o = nc.declare_dram_parameter("mu", [BPC, L], F32, isOutput=True)
    lv_o = nc.declare_dram_parameter("logvar", [BPC, L], F32, isOutput=True)

    nd_bounce = nc.dram_tensor("nd_bounce", [BPC, T], F32)

    x_flat = x_i[:, :, :].rearrange("b n h -> (b n) h")

    with tile.TileContext(nc) as tc, ExitStack() as ctx:
        singles = ctx.enter_context(tc.tile_pool(name="singles", bufs=1))
        xpool = ctx.enter_context(tc.tile_pool(name="xpool", bufs=3))
        xTpool = ctx.enter_context(tc.tile_pool(name="xTpool", bufs=2))
        scrpool = ctx.enter_context(tc.tile_pool(name="scr", bufs=3))
        epool = ctx.enter_context(tc.tile_pool(name="epool", bufs=4))
        chpool = ctx.enter_context(tc.tile_pool(name="chain", bufs=8))
        psT = ctx.enter_context(tc.tile_pool(name="psT", bufs=2, space="PSUM"))
        psA = ctx.enter_context(tc.tile_pool(name="psA", bufs=2, space="PSUM"))
        psC = ctx.enter_context(tc.tile_pool(name="psC", bufs=2, space="PSUM"))

        # ---------------- constants & weights ----------------
        identity = singles.tile([128, 128], F32)
        make_identity(nc, identity)

        ones = singles.tile([128, N], F32)
        nc.vector.memset(ones, 1.0)
        # masks[:, r, j] = 1.0 if j < 128*r + p else 0.0  (strict lower rows)
        masks = singles.tile([128, 8, N], F32)
        for r in range(8):
            nc.gpsimd.affine_select(
                out=masks[:, r, :],
                in_=ones,
                compare_op=mybir.AluOpType.is_gt,
                fill=0.0,
                base=128 * r,
                channel_multiplier=1,
                pattern=[[-1, N]],
            )

        w1sb = singles.tile([128, 4, H], F32)
        nc.sync.dma_start(out=w1sb, in_=w1_i[:, :].rearrange("(c p) h -> p c h", p=128))
        w2sb = singles.tile([128, 4, H], F32)
        nc.sync.dma_start(out=w2sb, in_=w2_i[:, :].rearrange("(c p) h -> p c h", p=128))
        # fold the 1/N mean into W2
        nc.scalar.mul(w2sb, w2sb, 1.0 / N)
        muwsb = singles.tile([128, 4, L], F32)
        nc.sync.dma_start(out=muwsb, in_=muw_i[:, :].rearrange("(c p) l -> p c l", p=128))
        lvwsb = singles.tile([128, 4, L], F32)
        nc.sync.dma_start(out=lvwsb, in_=lvw_i[:, :].rearrange("(c p) l -> p c l", p=128))
        zwsb = singles.tile([128, 2, H], F32)
        nc.sync.dma_start(out=zwsb, in_=zw_i[:, :].rearrange("(c p) h -> p c h", p=128))
        ndwsb = singles.tile([128, 4, T], F32)
        nc.sync.dma_start(out=ndwsb, in_=ndw_i[:, :].rearrange("(c p) t -> p c t", p=128))
        e1wsb = singles.tile([128, 8, H], F32)
        nc.sync.dma_start(out=e1wsb, in_=e1w_i[:, :].rearrange("(c p) h -> p c h", p=128))
        e2cols = singles.tile([128, 4], F32)
        nc.sync.dma_start(
            out=e2cols, in_=e2w_i[:, :].rearrange("(c p) o -> p (c o)", p=128)
        )

        b1sb = singles.tile([128, 4], F32)
        nc.sync.dma_start(out=b1sb, in_=b1_i[:].rearrange("(c p) -> p c", p=128))
        b2sb = singles.tile([128, 4], F32)
        nc.sync.dma_start(out=b2sb, in_=b2_i[:].rearrange("(c p) -> p c", p=128))
        mubsb = singles.tile([128, 2], F32)
        nc.sync.dma_start(out=mubsb, in_=mub_i[:].rearrange("(c p) -> p c", p=128))
        lvbsb = singles.tile([128, 2], F32)
        nc.sync.dma_start(out=lvbsb, in_=lvb_i[:].rearrange("(c p) -> p c", p=128))
        zbsb = singles.tile([128, 4], F32)
        nc.sync.dma_start(out=zbsb, in_=zb_i[:].rearrange("(c p) -> p c", p=128))
        e1bsb = singles.tile([128, 4], F32)
        nc.sync.dma_start(out=e1bsb, in_=e1b_i[:].rearrange("(c p) -> p c", p=128))
        ndbsb = singles.tile([128, 1], F32)
        nc.sync.dma_start(out=ndbsb[0:T, :], in_=ndb_i[:].rearrange("(t o) -> t o", o=1))
        e2bsb = singles.tile([128, 1], F32)
        nc.gpsimd.dma_start(
            out=e2bsb, in_=bass.AP(tensor=e2b_i, offset=0, ap=[[0, 128], [1, 1]])
        )

        # eps transposed: [128 l, c, b]
        epsT = singles.tile([128, 2, BPC], F32)
        for c in range(2):
            nc.sync.dma_start(
                out=epsT[:, c, :],
                in_=eps_i[:, ds(128 * c, 128)].rearrange("b l -> l b"),
            )

        # persistent accumulators / staging
        accall = singles.tile([128, NG, 4], F32)  # per (row-group, h-chunk) pool sums
        muall = singles.tile([128, 2, BPC], F32)  # mu^T staging (also chain input)
        lvall = singles.tile([128, 2, BPC], F32)
        ndall = singles.tile([128, 32], F32)  # [t, b] on partitions 0..31
        nc.vector.memset(ndall, 0.0)
        eall = singles.tile([128, BPC], F32)  # e on all partitions

        # ---------------- main loop ----------------
        for p in range(NPAIRS):
            # GEMM for the 4 row-groups of this batch pair
            for gg in range(4):
                g = 4 * p + gg
                xnat = xpool.tile([128, 4, H], F32)
                nc.sync.dma_start(
                    out=xnat,
                    in_=x_flat[ds(512 * g, 512), :].rearrange(
                        "(t p) f -> p t f", p=128
                    ),
                )
                xT = xTpool.tile([128, 4, 512], F32)  # [f_p, c, rows]
                for t in range(4):
                    pst = psT.tile([128, 4, 128], F32)
                    for c in range(4):
                        nc.tensor.transpose(
                            pst[:, c, :], xnat[:, t, ds(128 * c, 128)], identity
                        )
                    # one strided copy drops all 4 f-chunk blocks of row-tile t
                    nc.vector.tensor_copy(xT[:, :, ds(128 * t, 128)], pst)
                for m in range(4):
                    psa = psA.tile([128, 512], F32)
                    for c in range(4):
                        nc.tensor.matmul(
                            psa,
                            w1sb[:, c, ds(128 * m, 128)],
                            xT[:, c, :],
                            start=(c == 0),
                            stop=(c == 3),
                        )
                    scratch = scrpool.tile([128, 512], F32)
                    nc.scalar.activation(
                        out=scratch,
                        in_=psa,
                        func=AF.Relu,
                        bias=b1sb[:, ds(m, 1)],
                        scale=1.0,
                        accum_out=accall[:, g, ds(m, 1)],
                    )

            # ---- tiny decoder chain for batches (2p, 2p+1), rhs N=2 ----
            hp2 = chpool.tile([128, 4, 2], F32)
            for m in range(4):
                for j in range(2):
                    g0 = 4 * p + 2 * j
                    nc.vector.tensor_add(
                        hp2[:, m, ds(j, 1)],
                        accall[:, g0, ds(m, 1)],
                        accall[:, g0 + 1, ds(m, 1)],
                    )
            # hp_final^T = (W2/N)^T @ hp_sum + b2
            hpf = chpool.tile([128, 4, 2], F32)
            for m in range(4):
                ps = psC.tile([128, 2], F32, tag="chps")
                for k in range(4):
                    nc.tensor.matmul(
                        ps,
                        w2sb[:, k, ds(128 * m, 128)],
                        hp2[:, k, :],
                        start=(k == 0),
                        stop=(k == 3),
                    )
                nc.scalar.activation(
                    out=hpf[:, m, :], in_=ps, func=AF.Identity, bias=b2sb[:, ds(m, 1)]
                )
            # mu / logvar
            for m in range(2):
                ps = psC.tile([128, 2], F32, tag="chps")
                for k in range(4):
                    nc.tensor.matmul(
                        ps,
                        muwsb[:, k, ds(128 * m, 128)],
                        hpf[:, k, :],
                        start=(k == 0),
                        stop=(k == 3),
                    )
                nc.scalar.activation(
                    out=muall[:, m, ds(2 * p, 2)],
                    in_=ps,
                    func=AF.Identity,
                    bias=mubsb[:, ds(m, 1)],
                )
                ps2 = psC.tile([128, 2], F32, tag="chps")
                for k in range(4):
                    nc.tensor.matmul(
                        ps2,
                        lvwsb[:, k, ds(128 * m, 128)],
                        hpf[:, k, :],
                        start=(k == 0),
                        stop=(k == 3),
                    )
                nc.scalar.activation(
                    out=lvall[:, m, ds(2 * p, 2)],
                    in_=ps2,
                    func=AF.Identity,
                    bias=lvbsb[:, ds(m, 1)],
                )
            # z = mu + eps * exp(0.5 lv)
            zT = chpool.tile([128, 2, 2], F32)
            for m in range(2):
                tmp = chpool.tile([128, 2], F32)
                nc.scalar.activation(
                    out=tmp, in_=lvall[:, m, ds(2 * p, 2)], func=AF.Exp, scale=0.5
                )
                nc.vector.tensor_mul(tmp, tmp, epsT[:, m, ds(2 * p, 2)])
                nc.vector.tensor_add(zT[:, m, :], tmp, muall[:, m, ds(2 * p, 2)])
            # hd = relu(z_w^T z + z_b)
            hdT = chpool.tile([128, 4, 2], F32)
            for m in range(4):
                ps = psC.tile([128, 2], F32, tag="chps")
                for k in range(2):
                    nc.tensor.matmul(
                        ps,
                        zwsb[:, k, ds(128 * m, 128)],
                        zT[:, k, :],
                        start=(k == 0),
                        stop=(k == 1),
                    )
                nc.scalar.activation(
                    out=hdT[:, m, :], in_=ps, func=AF.Relu, bias=zbsb[:, ds(m, 1)]
                )
            # node logits row: nd^T [T, b]
            psn = psC.tile([128, 2], F32, tag="chps")
            for k in range(4):
                nc.tensor.matmul(
                    psn[0:T, :],
                    ndwsb[:, k, :],
                    hdT[:, k, :],
                    start=(k == 0),
                    stop=(k == 3),
                )
            nc.scalar.activation(
                out=ndall[0:T, ds(2 * p, 2)],
                in_=psn[0:T, :],
                func=AF.Identity,
                bias=ndbsb[0:T, :],
            )
            # he = relu(e1_w^T [hd;hd] + e1_b)
            heT = chpool.tile([128, 4, 2], F32)
            for m in range(4):
                ps = psC.tile([128, 2], F32, tag="chps")
                for k in range(8):
                    nc.tensor.matmul(
                        ps,
                        e1wsb[:, k, ds(128 * m, 128)],
                        hdT[:, k % 4, :],
                        start=(k == 0),
                        stop=(k == 7),
                    )
                nc.scalar.activation(
                    out=heT[:, m, :], in_=ps, func=AF.Relu, bias=e1bsb[:, ds(m, 1)]
                )
            # e = sigmoid(e2_w^T he + e2_b), then broadcast to all partitions
            pse = psC.tile([128, 2], F32, tag="chps")
            for k in range(4):
                nc.tensor.matmul(
                    pse[0:1, :],
                    e2cols[:, ds(k, 1)],
                    heT[:, k, :],
                    start=(k == 0),
                    stop=(k == 3),
                )
            erow = chpool.tile([128, 2], F32)
            nc.scalar.activation(
                out=erow[0:1, :],
                in_=pse[0:1, :],
                func=AF.Sigmoid,
                bias=e2bsb[0:1, :],
            )
            nc.gpsimd.partition_broadcast(eall[:, ds(2 * p, 2)], erow[0:1, :])

            # ---- edge_probs writes for this batch pair ----
            for j in range(2):
                bb = 2 * p + j
                for gg in range(4):
                    et = epool.tile([128, 2, N], F32)
                    for h2 in range(2):
                        r = 2 * gg + h2
                        nc.scalar.activation(
                            out=et[:, h2, :],
                            in_=masks[:, r, :],
                            func=AF.Copy,
                            scale=eall[:, ds(bb, 1)],
                        )
                    nc.sync.dma_start(
                        out=ep_o[bb, ds(256 * gg, 256), :].rearrange(
                            "(t p) c -> p t c", p=128
                        ),
                        in_=et,
                    )

        # ---------------- tail: node_logits + mu/logvar out ----------------
        ndR = chpool.tile([128, 32], F32)
        nc.vector.transpose(ndR[0:32, :], ndall[0:32, :])  # [b, t] rows 0..7
        nc.sync.dma_start(out=nd_bounce[:, :], in_=ndR[0:BPC, :])
        ndR128 = chpool.tile([128, T], F32)
        nc.gpsimd.dma_start(
            out=ndR128,
            in_=bass.AP(tensor=nd_bounce, offset=0, ap=[[T, BPC], [0, 16], [1, T]]),
        )
        nlx = singles.tile([128, 64, T], F32)
        nc.vector.tensor_copy(
            out=nlx,
            in_=bass.AP(
                tensor=ndR128.tensor,
                offset=ndR128.offset,
                ap=[ndR128.ap[0], [0, 64], [1, T]],
            ),
        )
        nc.sync.dma_start(
            out=nl_o[:, :, :].rearrange("b (pp rep) c -> (b pp) rep c", pp=16),
            in_=nlx,
        )
        for m in range(2):
            nc.sync.dma_start(
                out=mu_o[:, ds(128 * m, 128)].rearrange("b l -> l b"),
                in_=muall[:, m, :],
            )
            nc.sync.dma_start(
                out=lv_o[:, ds(128 * m, 128)].rearrange("b l -> l b"),
                in_=lvall[:, m, :],
            )

    nc.finalize()
    return nc


_NC = None


def kernel(**inputs):
    global _NC
    if _NC is None:
        _NC = _build()
    nc = _NC

    weights = {
        k: np.ascontiguousarray(np.asarray(inputs[k], dtype=np.float32))
        for k in (
            "gnn_w1", "gnn_b1", "gnn_w2", "gnn_b2",
            "mu_w", "mu_b", "lv_w", "lv_b",
            "z_w", "z_b", "nd_w", "nd_b",
            "e1_w", "e1_b", "e2_w", "e2_b",
        )
    }
    nf = np.asarray(inputs["node_features"], dtype=np.float32)
    eps = np.asarray(inputs["eps"], dtype=np.float32)

    in_maps = []
    for i in range(NCORES):
        m = dict(weights)
        m["node_features"] = np.ascontiguousarray(nf[i * BPC : (i + 1) * BPC])
        m["eps"] = np.ascontiguousarray(eps[i * BPC : (i + 1) * BPC])
        in_maps.append(m)

    res = run_bass_kernel_spmd(nc, in_maps, core_ids=list(range(NCORES)))
    outs = res.results
    node_logits = np.concatenate([o["node_logits"] for o in outs], axis=0)
    edge_probs = np.concatenate([o["edge_probs"] for o in outs], axis=0)
    mu = np.concatenate([o["mu"] for o in outs], axis=0)
    logvar = np.concatenate([o["logvar"] for o in outs], axis=0)
    return node_logits, edge_probs, mu, logvar


# revision 12
# speedup vs baseline: 1.7219x; 1.7219x over previous
"""Trainium2 Bass kernel for the DVAE problem.

Full-input contract: kernel(**inputs) takes the complete (unsharded) numpy
inputs and returns (node_logits, edge_probs, mu, logvar) as full numpy arrays.
Internally shards batch 64 -> 8 cores (data parallel), weights replicated.

Math (see reference):
  h      = relu(x @ W1 + b1) @ W2 + b2          [B,N,H]
  hp     = mean(h, axis=1)                      [B,H]
  mu/lv  = hp @ {mu_w,lv_w} + {mu_b,lv_b}       [B,L]
  z      = mu + eps * exp(0.5 lv)
  hd     = relu(z @ z_w + z_b)                  [B,H]
  nlrow  = hd @ nd_w + nd_b                     [B,T]  (broadcast over N)
  e      = sigmoid(relu([hd,hd] @ e1_w + e1_b) @ e2_w + e2_b)   [B]
  edge   = e[:,None,None] * strict_lower(N)     [B,N,N]

Key restructurings:
  - adj_matrix is unused by the math: never shipped or read.
  - mean commutes with the second GNN linear: pool relu(xW1+b1) first, then
    a tiny [B,H]x[H,H] matmul (W2 scaled by 1/N on device).
  - GEMM computed transposed ([h, rows] tiles) so bias is per-partition and
    the node-pool is a free ACT accum_out.
"""

import sys

sys.path.insert(0, "/opt/trn_rl_repo")

from contextlib import ExitStack

import numpy as np

import concourse.bass as bass
from concourse import bacc
import concourse.mybir as mybir
import concourse.tile as tile
from concourse.bass import ds
from concourse.bass_utils import run_bass_kernel_spmd
from concourse.masks import make_identity

F32 = mybir.dt.float32
F32R = mybir.dt.float32r
AF = mybir.ActivationFunctionType

B, N, H, L, T = 64, 1024, 512, 256, 32
NCORES = 8
BPC = B // NCORES  # batches per core = 8
ROWS = BPC * N  # 8192 rows per core
NG = ROWS // 512  # 16 row-groups of 512 rows
NPAIRS = BPC // 2  # chain granularity: 2 batches


def _r(ap):
    return ap.bitcast(F32R)


def _build():
    nc = bacc.Bacc(None, target_bir_lowering=False)

    x_i = nc.declare_dram_parameter("node_features", [BPC, N, H], F32, isOutput=False)
    eps_i = nc.declare_dram_parameter("eps", [BPC, L], F32, isOutput=False)
    w1_i = nc.declare_dram_parameter("gnn_w1", [H, H], F32, isOutput=False)
    b1_i = nc.declare_dram_parameter("gnn_b1", [H], F32, isOutput=False)
    w2_i = nc.declare_dram_parameter("gnn_w2", [H, H], F32, isOutput=False)
    b2_i = nc.declare_dram_parameter("gnn_b2", [H], F32, isOutput=False)
    muw_i = nc.declare_dram_parameter("mu_w", [H, L], F32, isOutput=False)
    mub_i = nc.declare_dram_parameter("mu_b", [L], F32, isOutput=False)
    lvw_i = nc.declare_dram_parameter("lv_w", [H, L], F32, isOutput=False)
    lvb_i = nc.declare_dram_parameter("lv_b", [L], F32, isOutput=False)
    zw_i = nc.declare_dram_parameter("z_w", [L, H], F32, isOutput=False)
    zb_i = nc.declare_dram_parameter("z_b", [H], F32, isOutput=False)
    ndw_i = nc.declare_dram_parameter("nd_w", [H, T], F32, isOutput=False)
    ndb_i = nc.declare_dram_parameter("nd_b", [T], F32, isOutput=False)
    e1w_i = nc.declare_dram_parameter("e1_w", [2 * H, H], F32, isOutput=False)
    e1b_i = nc.declare_dram_parameter("e1_b", [H], F32, isOutput=False)
    e2w_i = nc.declare_dram_parameter("e2_w", [H, 1], F32, isOutput=False)
    e2b_i = nc.declare_dram_parameter("e2_b", [1], F32, isOutput=False)

    nl_o = nc.declare_dram_parameter("node_logits", [BPC, N, T], F32, isOutput=True)
    ep_o = nc.declare_dram_parameter("edge_probs", [BPC, N, N], F32, isOutput=True)
    mu_o = nc.declare_dram_parameter("mu", [BPC, L], F32, isOutput=True)
    lv_o = nc.declare_dram_parameter("logvar", [BPC, L], F32, isOutput=True)

    nd_bounce = nc.dram_tensor("nd_bounce", [BPC, T], F32)

    x_flat = x_i[:, :, :].rearrange("b n h -> (b n) h")

    with tile.TileContext(nc) as tc, ExitStack() as ctx:
        singles = ctx.enter_context(tc.tile_pool(name="singles", bufs=1))
        xpool = ctx.enter_context(tc.tile_pool(name="xpool", bufs=3))
        xTpool = ctx.enter_context(tc.tile_pool(name="xTpool", bufs=2))
        scrpool = ctx.enter_context(tc.tile_pool(name="scr", bufs=3))
        epool = ctx.enter_context(tc.tile_pool(name="epool", bufs=4))
        chpool = ctx.enter_context(tc.tile_pool(name="chain", bufs=8))
        psT = ctx.enter_context(tc.tile_pool(name="psT", bufs=2, space="PSUM"))
        psA = ctx.enter_context(tc.tile_pool(name="psA", bufs=2, space="PSUM"))
        psC = ctx.enter_context(tc.tile_pool(name="psC", bufs=2, space="PSUM"))

        # ---------------- constants & weights ----------------
        identity = singles.tile([128, 128], F32)
        make_identity(nc, identity)

        ones = singles.tile([128, N], F32)
        nc.vector.memset(ones, 1.0)
        # masks[:, r, j] = 1.0 if j < 128*r + p else 0.0  (strict lower rows)
        masks = singles.tile([128, 8, N], F32)
        for r in range(8):
            nc.gpsimd.affine_select(
                out=masks[:, r, :],
                in_=ones,
                compare_op=mybir.AluOpType.is_gt,
                fill=0.0,
                base=128 * r,
                channel_multiplier=1,
                pattern=[[-1, N]],
            )

        w1sb = singles.tile([128, 4, H], F32R)
        nc.gpsimd.dma_start(out=w1sb, in_=w1_i[:, :].rearrange("(c p) h -> p c h", p=128))
        # 1/N mean folded into the relu (scale=1/N; relu is pos.-homogeneous)
        w2sb = singles.tile([128, 4, H], F32R)
        nc.gpsimd.dma_start(out=w2sb, in_=w2_i[:, :].rearrange("(c p) h -> p c h", p=128))
        muwsb = singles.tile([128, 4, L], F32R)
        nc.gpsimd.dma_start(out=muwsb, in_=muw_i[:, :].rearrange("(c p) l -> p c l", p=128))
        lvwsb = singles.tile([128, 4, L], F32R)
        nc.gpsimd.dma_start(out=lvwsb, in_=lvw_i[:, :].rearrange("(c p) l -> p c l", p=128))
        zwsb = singles.tile([128, 2, H], F32R)
        nc.gpsimd.dma_start(out=zwsb, in_=zw_i[:, :].rearrange("(c p) h -> p c h", p=128))
        ndwsb = singles.tile([128, 4, T], F32R)
        nc.gpsimd.dma_start(out=ndwsb, in_=ndw_i[:, :].rearrange("(c p) t -> p c t", p=128))
        e1wsb = singles.tile([128, 8, H], F32R)
        nc.gpsimd.dma_start(out=e1wsb, in_=e1w_i[:, :].rearrange("(c p) h -> p c h", p=128))
        e2cols = singles.tile([128, 4], F32R)
        nc.gpsimd.dma_start(
            out=e2cols, in_=e2w_i[:, :].rearrange("(c p) o -> p (c o)", p=128)
        )

        b1sb = singles.tile([128, 4], F32)
        nc.sync.dma_start(out=b1sb, in_=b1_i[:].rearrange("(c p) -> p c", p=128))
        nc.scalar.mul(b1sb, b1sb, 1.0 / N)
        b2sb = singles.tile([128, 4], F32)
        nc.sync.dma_start(out=b2sb, in_=b2_i[:].rearrange("(c p) -> p c", p=128))
        mubsb = singles.tile([128, 2], F32)
        nc.sync.dma_start(out=mubsb, in_=mub_i[:].rearrange("(c p) -> p c", p=128))
        lvbsb = singles.tile([128, 2], F32)
        nc.sync.dma_start(out=lvbsb, in_=lvb_i[:].rearrange("(c p) -> p c", p=128))
        zbsb = singles.tile([128, 4], F32)
        nc.sync.dma_start(out=zbsb, in_=zb_i[:].rearrange("(c p) -> p c", p=128))
        e1bsb = singles.tile([128, 4], F32)
        nc.sync.dma_start(out=e1bsb, in_=e1b_i[:].rearrange("(c p) -> p c", p=128))
        ndbsb = singles.tile([128, 1], F32)
        nc.sync.dma_start(out=ndbsb[0:T, :], in_=ndb_i[:].rearrange("(t o) -> t o", o=1))
        e2bsb = singles.tile([128, 1], F32)
        nc.gpsimd.dma_start(
            out=e2bsb, in_=bass.AP(tensor=e2b_i, offset=0, ap=[[0, 128], [1, 1]])
        )

        # eps transposed: [128 l, c, b]
        epsT = singles.tile([128, 2, BPC], F32)
        for c in range(2):
            nc.sync.dma_start(
                out=epsT[:, c, :],
                in_=eps_i[:, ds(128 * c, 128)].rearrange("b l -> l b"),
            )

        # persistent accumulators / staging
        accall = singles.tile([128, NG, 4], F32)  # per (row-group, h-chunk) pool sums
        muall = singles.tile([128, 2, BPC], F32)  # mu^T staging (also chain input)
        lvall = singles.tile([128, 2, BPC], F32)
        ndall = singles.tile([128, 32], F32)  # [t, b] on partitions 0..31
        nc.vector.memset(ndall, 0.0)
        eall = singles.tile([128, BPC], F32)  # e on all partitions

        # ---------------- main loop ----------------
        for p in range(NPAIRS):
            # GEMM for the 4 row-groups of this batch pair
            for gg in range(4):
                g = 4 * p + gg
                xnat = xpool.tile([128, 4, H], F32)
                nc.sync.dma_start(
                    out=xnat,
                    in_=x_flat[ds(512 * g, 512), :].rearrange(
                        "(t p) f -> p t f", p=128
                    ),
                )
                xT = xTpool.tile([128, 4, 512], F32R)  # [f_p, c, rows], fp32r-rounded
                for t in range(4):
                    pst = psT.tile([128, 4, 128], F32)
                    for c in range(4):
                        nc.tensor.transpose(
                            pst[:, c, :], xnat[:, t, ds(128 * c, 128)], identity
                        )
                    # one strided copy drops all 4 f-chunk blocks of row-tile t
                    nc.vector.tensor_copy(xT[:, :, ds(128 * t, 128)], pst)
                for m in range(4):
                    psa = psA.tile([128, 512], F32)
                    for c in range(4):
                        nc.tensor.matmul(
                            psa,
                            w1sb[:, c, ds(128 * m, 128)],
                            xT[:, c, :],
                            start=(c == 0),
                            stop=(c == 3),
                        )
                    scratch = scrpool.tile([128, 512], F32)
                    nc.scalar.activation(
                        out=scratch,
                        in_=psa,
                        func=AF.Relu,
                        bias=b1sb[:, ds(m, 1)],
                        scale=1.0 / N,
                        accum_out=accall[:, g, ds(m, 1)],
                    )

            # ---- tiny decoder chain for batches (2p, 2p+1), rhs N=2 ----
            hp2 = chpool.tile([128, 4, 2], F32R)
            for m in range(4):
                for j in range(2):
                    g0 = 4 * p + 2 * j
                    nc.vector.tensor_add(
                        hp2[:, m, ds(j, 1)],
                        accall[:, g0, ds(m, 1)],
                        accall[:, g0 + 1, ds(m, 1)],
                    )
            # hp_final^T = (W2/N)^T @ hp_sum + b2
            hpf = chpool.tile([128, 4, 2], F32R)
            for m in range(4):
                ps = psC.tile([128, 2], F32, tag="chps")
                for k in range(4):
                    nc.tensor.matmul(
                        ps,
                        w2sb[:, k, ds(128 * m, 128)],
                        hp2[:, k, :],
                        start=(k == 0),
                        stop=(k == 3),
                    )
                nc.scalar.activation(
                    out=hpf[:, m, :], in_=ps, func=AF.Identity, bias=b2sb[:, ds(m, 1)]
                )
            # mu / logvar
            for m in range(2):
                ps = psC.tile([128, 2], F32, tag="chps")
                for k in range(4):
                    nc.tensor.matmul(
                        ps,
                        muwsb[:, k, ds(128 * m, 128)],
                        hpf[:, k, :],
                        start=(k == 0),
                        stop=(k == 3),
                    )
                nc.scalar.activation(
                    out=muall[:, m, ds(2 * p, 2)],
                    in_=ps,
                    func=AF.Identity,
                    bias=mubsb[:, ds(m, 1)],
                )
                ps2 = psC.tile([128, 2], F32, tag="chps")
                for k in range(4):
                    nc.tensor.matmul(
                        ps2,
                        lvwsb[:, k, ds(128 * m, 128)],
                        hpf[:, k, :],
                        start=(k == 0),
                        stop=(k == 3),
                    )
                nc.scalar.activation(
                    out=lvall[:, m, ds(2 * p, 2)],
                    in_=ps2,
                    func=AF.Identity,
                    bias=lvbsb[:, ds(m, 1)],
                )
            # z = mu + eps * exp(0.5 lv)
            zT = chpool.tile([128, 2, 2], F32R)
            for m in range(2):
                tmp = chpool.tile([128, 2], F32)
                nc.scalar.activation(
                    out=tmp, in_=lvall[:, m, ds(2 * p, 2)], func=AF.Exp, scale=0.5
                )
                nc.vector.tensor_mul(tmp, tmp, epsT[:, m, ds(2 * p, 2)])
                nc.vector.tensor_add(zT[:, m, :], tmp, muall[:, m, ds(2 * p, 2)])
            # hd = relu(z_w^T z + z_b)
            hdT = chpool.tile([128, 4, 2], F32R)
            for m in range(4):
                ps = psC.tile([128, 2], F32, tag="chps")
                for k in range(2):
                    nc.tensor.matmul(
                        ps,
                        zwsb[:, k, ds(128 * m, 128)],
                        zT[:, k, :],
                        start=(k == 0),
                        stop=(k == 1),
                    )
                nc.scalar.activation(
                    out=hdT[:, m, :], in_=ps, func=AF.Relu, bias=zbsb[:, ds(m, 1)]
                )
            # node logits row: nd^T [T, b]
            psn = psC.tile([128, 2], F32, tag="chps")
            for k in range(4):
                nc.tensor.matmul(
                    psn[0:T, :],
                    ndwsb[:, k, :],
                    hdT[:, k, :],
                    start=(k == 0),
                    stop=(k == 3),
                )
            nc.scalar.activation(
                out=ndall[0:T, ds(2 * p, 2)],
                in_=psn[0:T, :],
                func=AF.Identity,
                bias=ndbsb[0:T, :],
            )
            # he = relu(e1_w^T [hd;hd] + e1_b)
            heT = chpool.tile([128, 4, 2], F32R)
            for m in range(4):
                ps = psC.tile([128, 2], F32, tag="chps")
                for k in range(8):
                    nc.tensor.matmul(
                        ps,
                        e1wsb[:, k, ds(128 * m, 128)],
                        hdT[:, k % 4, :],
                        start=(k == 0),
                        stop=(k == 7),
                    )
                nc.scalar.activation(
                    out=heT[:, m, :], in_=ps, func=AF.Relu, bias=e1bsb[:, ds(m, 1)]
                )
            # e = sigmoid(e2_w^T he + e2_b), then broadcast to all partitions
            pse = psC.tile([128, 2], F32, tag="chps")
            for k in range(4):
                nc.tensor.matmul(
                    pse[0:1, :],
                    e2cols[:, ds(k, 1)],
                    heT[:, k, :],
                    start=(k == 0),
                    stop=(k == 3),
                )
            erow = chpool.tile([128, 2], F32)
            nc.scalar.activation(
                out=erow[0:1, :],
                in_=pse[0:1, :],
                func=AF.Sigmoid,
                bias=e2bsb[0:1, :],
            )
            nc.gpsimd.partition_broadcast(eall[:, ds(2 * p, 2)], erow[0:1, :])

            # ---- edge_probs writes for this batch pair ----
            for j in range(2):
                bb = 2 * p + j
                for gg in range(4):
                    et = epool.tile([128, 2, N], F32)
                    for h2 in range(2):
                        r = 2 * gg + h2
                        nc.vector.tensor_scalar_mul(
                            et[:, h2, :], masks[:, r, :], eall[:, ds(bb, 1)]
                        )
                    nc.sync.dma_start(
                        out=ep_o[bb, ds(256 * gg, 256), :].rearrange(
                            "(t p) c -> p t c", p=128
                        ),
                        in_=et,
                    )

        # ---------------- tail: node_logits + mu/logvar out ----------------
        ndR = chpool.tile([128, 32], F32)
        nc.vector.transpose(ndR[0:32, :], ndall[0:32, :])  # [b, t] rows 0..7
        nc.sync.dma_start(out=nd_bounce[:, :], in_=ndR[0:BPC, :])
        ndR128 = chpool.tile([128, T], F32)
        nc.gpsimd.dma_start(
            out=ndR128,
            in_=bass.AP(tensor=nd_bounce, offset=0, ap=[[T, BPC], [0, 16], [1, T]]),
        )
        nlx = singles.tile([128, 64, T], F32)
        nc.vector.tensor_copy(
            out=nlx,
            in_=bass.AP(
                tensor=ndR128.tensor,
                offset=ndR128.offset,
                ap=[ndR128.ap[0], [0, 64], [1, T]],
            ),
        )
        nc.sync.dma_start(
            out=nl_o[:, :, :].rearrange("b (pp rep) c -> (b pp) rep c", pp=16),
            in_=nlx,
        )
        for m in range(2):
            nc.sync.dma_start(
                out=mu_o[:, ds(128 * m, 128)].rearrange("b l -> l b"),
                in_=muall[:, m, :],
            )
            nc.sync.dma_start(
                out=lv_o[:, ds(128 * m, 128)].rearrange("b l -> l b"),
                in_=lvall[:, m, :],
            )

    nc.finalize()
    return nc


_NC = None


def kernel(**inputs):
    global _NC
    if _NC is None:
        _NC = _build()
    nc = _NC

    weights = {
        k: np.ascontiguousarray(np.asarray(inputs[k], dtype=np.float32))
        for k in (
            "gnn_w1", "gnn_b1", "gnn_w2", "gnn_b2",
            "mu_w", "mu_b", "lv_w", "lv_b",
            "z_w", "z_b", "nd_w", "nd_b",
            "e1_w", "e1_b", "e2_w", "e2_b",
        )
    }
    nf = np.asarray(inputs["node_features"], dtype=np.float32)
    eps = np.asarray(inputs["eps"], dtype=np.float32)

    in_maps = []
    for i in range(NCORES):
        m = dict(weights)
        m["node_features"] = np.ascontiguousarray(nf[i * BPC : (i + 1) * BPC])
        m["eps"] = np.ascontiguousarray(eps[i * BPC : (i + 1) * BPC])
        in_maps.append(m)

    res = run_bass_kernel_spmd(nc, in_maps, core_ids=list(range(NCORES)))
    outs = res.results
    node_logits = np.concatenate([o["node_logits"] for o in outs], axis=0)
    edge_probs = np.concatenate([o["edge_probs"] for o in outs], axis=0)
    mu = np.concatenate([o["mu"] for o in outs], axis=0)
    logvar = np.concatenate([o["logvar"] for o in outs], axis=0)
    return node_logits, edge_probs, mu, logvar
